# revision 63
# baseline (speedup 1.0000x reference)
"""NemotronH Mamba2 decoder layer on 8 Trainium2 cores (Bass/Tile).

Sharding: tensor-parallel over the 8 SSM groups (1 group = 8 heads / core).
in_proj, conv, A, D, dt_bias sharded along d_inner; out_proj sharded along its
input dim with a pipelined bf16 ReduceScatter (gate-norm sumsq folded in as an
extra RS column); RMSNorm replicated.

Pipeline per 512-token group g: pass A -> in_proj -> conv -> dt prep; scan
chunks 4g..4g+3 overlap in_proj of g+1; out_proj + RS + final scale for group
g are emitted right after scan chunk 4g+3 so the collectives overlap the scan.

Scan (chunked SSD, Q=128) uses wide ops: the per-head decay matrix exponent
c_t - c_s + ldt_s is built for all 8 heads in one PSUM tile via select-matmuls
(3-level bf16 hi/lo/lolo splits keep it exact); the causal mask is an additive
-1e30 const; G = B^T C is shared via stride-0 broadcast APs; D is folded in via
diagonal matmuls into the Y PSUM; v = (Y + Dx) * silu(z) is one strided mul.
"""
import contextlib
import os
import sys
import types

import numpy as np
import ml_dtypes

# --- axon NTFF profile hook shim (lets trace=True work in this container) ---
try:
    import antenv
    if "antenv.axon_hooks" not in sys.modules:
        try:
            from trn_agent_boot.trn_boot import _ntff_profile_via_ctypes
            _hooks = types.ModuleType("antenv.axon_hooks")
            _hook = _ntff_profile_via_ctypes("/opt/axon/libaxon_pjrt.so")
            _hooks.get_axon_ntff_profile_hook = lambda: _hook
            sys.modules["antenv.axon_hooks"] = _hooks
            antenv.axon_hooks = _hooks
        except Exception:
            pass
except Exception:
    pass

import concourse.bass as bass
import concourse.bacc as bacc
import concourse.tile as tile
import concourse.mybir as mybir
import concourse.bass_utils as bass_utils

bass_utils.upload_artifacts = lambda tmpdir: tmpdir  # no S3 in-container

FP32 = mybir.dt.float32
BF16 = mybir.dt.bfloat16
AF = mybir.ActivationFunctionType
ALU = mybir.AluOpType

NCORES = 8
BT = 2048        # B*L tokens
DM = 2048        # model dim
DI = 512         # d_inner slice per core (8 heads x 64)
NH = 8           # heads per core
PD = 64          # head dim
Q = 128          # scan chunk length
NCH = BT // Q    # 16 chunks
NGRP = 4         # token groups for pipelining
GSZ = BT // NGRP # 512
EPS = 1e-5
CVC = 518        # conv buffer cols: 3 history + 512 + 3 slack
RSW = 2056       # RS row width: 2048 out + 1 sumsq + 7 pad

_BUILT = None
LAST_RESULTS = None


def _bc(ap, n):
    """Insert a stride-0 middle free dim of size n: [P, W] -> [P, n, W]."""
    return bass.AP(ap.tensor, ap.offset, [ap.ap[0], [0, n]] + ap.ap[1:])


class _StopBuild(Exception):
    pass
_STOP = os.environ.get("K_STOP", "full")  # passa|inproj|nocc|full
_DBG = os.environ.get("K_DBG") == "1"


def _build():
    nc = bacc.Bacc("TRN2", target_bir_lowering=False, debug=False,
                   num_devices=NCORES)

    def inp(name, shape, dt):
        return nc.dram_tensor(name, shape, dt, kind="ExternalInput").ap()

    hid = inp("hid", [BT, DM], FP32)
    res = inp("res", [BT, DM], FP32)
    w_in_t = inp("w_in_t", [DM, 1288], BF16)
    w_out_t = inp("w_out_t", [DI, DM], BF16)
    a_col = inp("a_col", [NH, 1], FP32)
    dtb_col = inp("dtb_col", [NH, 1], FP32)
    convw = inp("convw", [128, 24], FP32)
    convb = inp("convb", [128, 6], FP32)
    ones_col_bf = inp("ones_col_bf", [128, 1], BF16)
    i_bf = inp("i_bf", [128, 128], BF16)
    ones3 = inp("ones3", [3, 128], BF16)
    sel24n = inp("sel24n", [24, NH * Q], BF16)
    sel16p = inp("sel16p", [16, NH * Q], BF16)
    msk_l = inp("msk_l", [128, 128], BF16)
    msk_r = inp("msk_r", [128, NH * Q], BF16)
    wsel = inp("wsel", [NH, DI], BF16)
    diag4 = inp("diag4", [128, 512], BF16)

    new_res = nc.dram_tensor("new_res", [BT, DM], BF16,
                             kind="ExternalOutput").ap()
    out_rs = nc.dram_tensor("out_rs", [256, DM], BF16,
                            kind="ExternalOutput").ap()

    rg = [list(range(NCORES))]

    if _DBG:
        dbg_mtp = nc.dram_tensor("dbg_mtp", [128, NH * Q], BF16,
                                 kind="ExternalOutput").ap()
        dbg_cec = nc.dram_tensor("dbg_cec", [128, NH * Q], BF16,
                                 kind="ExternalOutput").ap()
        dbg_ssb = nc.dram_tensor("dbg_ssb", [128, 512], FP32,
                                 kind="ExternalOutput").ap()
        dbg_xtk = nc.dram_tensor("dbg_xtk", [128, 512], BF16,
                                 kind="ExternalOutput").ap()
        dbg_v = nc.dram_tensor("dbg_v", [128, 4 * BT], BF16,
                               kind="ExternalOutput").ap()

    with tile.TileContext(nc) as tc:
        try:
            with contextlib.ExitStack() as stack:
                ec = stack.enter_context
                cpool = ec(tc.tile_pool(name="const", bufs=1))
                dram = ec(tc.tile_pool(name="dram", bufs=1, space="DRAM"))
                mid = ec(tc.tile_pool(name="mid", bufs=1))
                wpool = ec(tc.tile_pool(name="wpool", bufs=1))
                convp = ec(tc.tile_pool(name="convp", bufs=1))
                pa = ec(tc.tile_pool(name="pa", bufs=2))
                ip = ec(tc.tile_pool(name="ip", bufs=2))
                ipx = ec(tc.tile_pool(name="ipx", bufs=1))
                sc = ec(tc.tile_pool(name="sc", bufs=2))
                sc1 = ec(tc.tile_pool(name="sc1", bufs=1))
                scst = ec(tc.tile_pool(name="scst", bufs=2))
                op = ec(tc.tile_pool(name="op", bufs=1))
                fin = ec(tc.tile_pool(name="fin", bufs=1))
                pmm = ec(tc.tile_pool(name="pmm", bufs=2, space="PSUM"))
                pwide = ec(tc.tile_pool(name="pwide", bufs=2, space="PSUM"))
                ptrp = ec(tc.tile_pool(name="ptrp", bufs=2, space="PSUM"))
                pys = ec(tc.tile_pool(name="pys", bufs=2, space="PSUM"))

                # ---------------- constants ----------------
                c_ones_col = cpool.tile([128, 1], BF16)
                nc.sync.dma_start(c_ones_col[:], ones_col_bf[:])
                c_ibf = cpool.tile([128, 128], BF16)
                nc.sync.dma_start(c_ibf[:], i_bf[:])
                c_acol = cpool.tile([NH, 1], FP32)
                nc.sync.dma_start(c_acol[:], a_col[:])
                c_dtb = cpool.tile([NH, 1], FP32)
                nc.sync.dma_start(c_dtb[:], dtb_col[:])
                c_convw = cpool.tile([128, 24], FP32)
                nc.sync.dma_start(c_convw[:], convw[:])
                c_convb = cpool.tile([128, 6], FP32)
                nc.sync.dma_start(c_convb[:], convb[:])
                c_ones3 = cpool.tile([3, 128], BF16)
                nc.sync.dma_start(c_ones3[:], ones3[:])
                c_sel24n = cpool.tile([24, NH * Q], BF16)
                nc.sync.dma_start(c_sel24n[:], sel24n[:])
                c_sel16p = cpool.tile([16, NH * Q], BF16)
                nc.sync.dma_start(c_sel16p[:], sel16p[:])
                c_mskl = cpool.tile([128, 128], BF16)
                nc.sync.dma_start(c_mskl[:], msk_l[:])
                c_mskr = cpool.tile([128, NH * Q], BF16)
                nc.sync.dma_start(c_mskr[:], msk_r[:])
                c_wsel = cpool.tile([NH, DI], BF16)
                nc.sync.dma_start(c_wsel[:], wsel[:])
                c_diag4 = cpool.tile([128, 512], BF16)
                nc.sync.dma_start(c_diag4[:], diag4[:])
                z8 = cpool.tile([NH, Q], FP32)
                nc.vector.memset(z8[:], 0.0)
                c_eps = cpool.tile([128, 1], FP32)
                nc.vector.memset(c_eps[:], EPS)

                xs_dram = dram.tile([BT, DM], BF16)
                rs_in = [dram.tile([GSZ, RSW], BF16, name=f"rsin{g}")
                         for g in range(NGRP)]
                rs_out = [dram.tile([64, RSW], BF16, name=f"rsout{g}")
                          for g in range(NGRP)]

                # ---------------- weight prefetch ----------------
                wt = [wpool.tile([128, 1288], BF16, name=f"wt{k}")
                      for k in range(16)]
                for k in range(16):
                    nc.sync.dma_start(wt[k][:], w_in_t[k * 128:(k + 1) * 128, :])
                wo = [wpool.tile([128, DM], BF16, name=f"wo{k}")
                      for k in range(4)]
                for k in range(4):
                    nc.sync.dma_start(wo[k][:], w_out_t[k * 128:(k + 1) * 128, :])

                # ---------------- mid-life activations ----------------
                xall = mid.tile([128, 4 * BT], BF16)     # x, then v in place
                szall = mid.tile([128, 4 * BT], BF16)    # silu(z)
                xbcb = mid.tile([128, BT], BF16)         # B (feat-major)
                xbcc = mid.tile([128, BT], BF16)         # C (feat-major)
                ldt = mid.tile([NH, BT], FP32)
                a_row = mid.tile([NH, BT], FP32)
                rsq = mid.tile([128, 16], FP32)

                cvb = [convp.tile([128, CVC], BF16, name=f"cvb{i}")
                       for i in range(6)]
                for i in range(6):
                    nc.vector.memset(cvb[i][:, 0:3], 0.0)

                xav = xall[:].rearrange("p (a t) -> p a t", a=4)
                szv = szall[:].rearrange("p (a t) -> p a t", a=4)

                # ============ pipelined groups: passA -> in_proj -> conv ====
                for g in range(NGRP):
                    gc = slice(g * GSZ, (g + 1) * GSZ)
                    # ---- pass A (fused): h, new_res, rsqrt, xs ----
                    HW = DM // 2
                    for i in range(4):
                        ti = 4 * g + i
                        rows = slice(ti * 128, (ti + 1) * 128)
                        hts, sss = [], []
                        for hf in range(2):
                            cl = slice(hf * HW, (hf + 1) * HW)
                            th = pa.tile([128, HW], FP32, tag="hid")
                            tr = pa.tile([128, HW], FP32, tag="res")
                            nc.sync.dma_start(th[:], hid[rows, cl])
                            nc.sync.dma_start(tr[:], res[rows, cl])
                            hsum = pa.tile([128, HW], BF16, tag=f"h{hf}")
                            nc.vector.tensor_add(hsum[:], th[:], tr[:])
                            nc.sync.dma_start(new_res[rows, cl], hsum[:])
                            sq = pa.tile([128, HW], BF16, tag="xs")
                            ss = pa.tile([128, 1], FP32, tag=f"ss{hf}")
                            nc.scalar.activation(sq[:], hsum[:], AF.Square,
                                                 accum_out=ss[:])
                            hts.append(hsum)
                            sss.append(ss)
                        ssf = pa.tile([128, 1], FP32, tag="ssf")
                        nc.vector.tensor_add(ssf[:], sss[0][:], sss[1][:])
                        ln = pa.tile([128, 1], FP32, tag="ln")
                        nc.scalar.activation(ln[:], ssf[:], AF.Ln,
                                             scale=1.0 / DM, bias=c_eps[:])
                        nc.scalar.activation(rsq[:, ti:ti + 1], ln[:], AF.Exp,
                                             scale=-0.5)
                        for hf in range(2):
                            cl = slice(hf * HW, (hf + 1) * HW)
                            xsb = pa.tile([128, HW], BF16, tag="xs")
                            nc.vector.tensor_scalar_mul(xsb[:], hts[hf][:],
                                                        rsq[:, ti:ti + 1])
                            nc.sync.dma_start(xs_dram[rows, cl], xsb[:])

                    # ---- x^T k-tiles for this group ----
                    xt = [ipx.tile([128, GSZ], BF16, tag=f"xt{k}",
                                   name=f"xt{g}_{k}") for k in range(16)]
                    for k in range(16):
                        nc.sync.dma_start_transpose(
                            xt[k][:], xs_dram[gc, k * 128:(k + 1) * 128])

                    if _STOP == "passa":
                        continue
                    # ---- in_proj: M-tiles 0-3 x, 4 B, 5 C, 6-9 z, 10 dt ----
                    dt_raw = ip.tile([NH, GSZ], FP32, tag="dtraw")
                    for m in range(11):
                        mrows = 8 if m == 10 else 128
                        ps = pmm.tile([128, GSZ], FP32, tag="mm")
                        for k in range(16):
                            nc.tensor.matmul(
                                ps[0:mrows, :],
                                wt[k][:, m * 128:m * 128 + mrows],
                                xt[k][:],
                                start=(k == 0), stop=(k == 15))
                        if m < 6:
                            nc.scalar.copy(cvb[m][:, 3:3 + GSZ], ps[:, :])
                        elif m < 10:
                            nc.scalar.activation(
                                szall[:, (m - 6) * BT + g * GSZ:
                                      (m - 6) * BT + (g + 1) * GSZ],
                                ps[:, :], AF.Silu)
                        else:
                            nc.scalar.copy(dt_raw[:], ps[0:8, :])

                    # ---- conv + silu (bf16 chain) ----
                    for i in range(6):
                        cw = [c_convw[:, i * 4 + k:i * 4 + k + 1]
                              for k in range(4)]
                        t0 = ip.tile([128, GSZ], BF16, tag="cv")
                        nc.vector.tensor_scalar_mul(t0[:], cvb[i][:, 0:GSZ],
                                                    cw[0])
                        t1 = ip.tile([128, GSZ], BF16, tag="cv")
                        nc.vector.scalar_tensor_tensor(
                            t1[:], cvb[i][:, 1:1 + GSZ], cw[1], t0[:],
                            ALU.mult, ALU.add)
                        t2 = ip.tile([128, GSZ], BF16, tag="cv")
                        nc.vector.scalar_tensor_tensor(
                            t2[:], cvb[i][:, 2:2 + GSZ], cw[2], t1[:],
                            ALU.mult, ALU.add)
                        t3 = ip.tile([128, GSZ], BF16, tag="cv")
                        nc.vector.scalar_tensor_tensor(
                            t3[:], cvb[i][:, 3:3 + GSZ], cw[3], t2[:],
                            ALU.mult, ALU.add)
                        if i < 4:
                            dst = xall[:, i * BT + g * GSZ:
                                       i * BT + (g + 1) * GSZ]
                        elif i == 4:
                            dst = xbcb[:, gc]
                        else:
                            dst = xbcc[:, gc]
                        nc.scalar.activation(dst, t3[:], AF.Silu,
                                             bias=c_convb[:, i:i + 1])
                        # roll conv history (zero across the batch boundary)
                        if g == 1:
                            nc.vector.memset(cvb[i][:, 0:3], 0.0)
                        else:
                            nc.vector.tensor_copy(cvb[i][:, 0:3],
                                                  cvb[i][:, GSZ:GSZ + 3])

                    # ---- dt prep for this group ----
                    e1 = ip.tile([NH, GSZ], FP32, tag="dtraw")
                    nc.scalar.activation(e1[:], dt_raw[:], AF.Exp,
                                         bias=c_dtb[:])
                    e2 = ip.tile([NH, GSZ], FP32, tag="dtraw")
                    nc.vector.tensor_scalar_add(e2[:], e1[:], 1.0)
                    dt_v = ip.tile([NH, GSZ], FP32, tag="dtraw")
                    nc.scalar.activation(dt_v[:], e2[:], AF.Ln)
                    nc.scalar.activation(ldt[:, gc], dt_v[:], AF.Ln)
                    nc.vector.tensor_scalar_mul(a_row[:, gc], dt_v[:],
                                                c_acol[:])

                # ---------------- scan + interleaved out_proj/RS ------------
                if _STOP in ("passa", "inproj"):
                    raise _StopBuild()
                s_bf_prev = None
                s_sb_prev = None
                for ci in range(NCH):
                    cols = slice(ci * Q, (ci + 1) * Q)
                    first = (ci % 8 == 0)

                    # cumsum of A*dt, wrow = exp(ldt - c + c_end)
                    c_t = sc.tile([NH, Q], FP32, tag="c")
                    nc.vector.tensor_tensor_scan(
                        c_t[:], a_row[:, cols], z8[:], 0.0, ALU.add, ALU.add)
                    lc = sc.tile([NH, Q], FP32, tag="lc")
                    nc.vector.tensor_sub(lc[:], ldt[:, cols], c_t[:])
                    wrow = sc.tile([NH, Q], FP32, tag="lc")
                    nc.scalar.activation(wrow[:], lc[:], AF.Exp,
                                         bias=c_t[:, Q - 1:Q])
                    wrowbf = sc.tile([NH, Q], BF16, tag="btk")
                    nc.vector.tensor_copy(wrowbf[:], wrow[:])

                    # 3-level split of c + 2-level split of ldt (exact in sum)
                    # (compute engines can't write partition offsets that are
                    #  not 32-aligned -> stage split rows via SBUF-SBUF DMA)
                    cl3 = sc.tile([24, Q], BF16, tag="cl3")
                    dl2 = sc.tile([16, Q], BF16, tag="cl3")
                    r1 = sc.tile([NH, Q], FP32, tag="lc")
                    r2 = sc.tile([NH, Q], FP32, tag="lc")
                    clo = sc1.tile([NH, Q], BF16, tag="clo")
                    cll = sc1.tile([NH, Q], BF16, tag="cll")
                    nc.vector.tensor_copy(cl3[0:8, :], c_t[:])
                    nc.vector.tensor_sub(r1[:], c_t[:], cl3[0:8, :])
                    nc.vector.tensor_copy(clo[:], r1[:])
                    nc.vector.tensor_sub(r2[:], r1[:], clo[:])
                    nc.vector.tensor_copy(cll[:], r2[:])
                    nc.sync.dma_start(cl3[8:16, :], clo[:])
                    nc.sync.dma_start(cl3[16:24, :], cll[:])
                    nc.vector.tensor_copy(dl2[0:8, :], ldt[:, cols])
                    r3 = sc1.tile([NH, Q], FP32, tag="r1")
                    nc.vector.tensor_sub(r3[:], ldt[:, cols], dl2[0:8, :])
                    dlo = sc1.tile([NH, Q], BF16, tag="clo")
                    nc.vector.tensor_copy(dlo[:], r3[:])
                    nc.sync.dma_start(dl2[8:16, :], dlo[:])

                    crow3 = sc1.tile([3, NH * Q], BF16, tag="crow3")
                    nc.sync.dma_start(crow3[0:1, :], cl3[0:8, :])
                    nc.sync.dma_start(crow3[1:2, :], clo[:])
                    nc.sync.dma_start(crow3[2:3, :], cll[:])

                    # G = B^T C (shared across heads)
                    gmp = pwide.tile([128, 512], FP32, tag="pw")
                    nc.tensor.matmul(gmp[:, 0:Q], xbcb[:, cols],
                                     xbcc[:, cols], start=True, stop=True)
                    gm = sc1.tile([128, Q], BF16, tag="gm")
                    nc.vector.tensor_copy(gm[:], gmp[:, 0:Q])

                    mtp = sc1.tile([128, NH * Q], BF16, tag="mtp")
                    mtp3 = mtp[:].rearrange("p (r t) -> p r t", r=NH)
                    cec = sc1.tile([128, NH * Q], BF16, tag="cec")
                    cec3 = cec[:].rearrange("p (r t) -> p r t", r=NH)
                    ets = []
                    for half in range(2):
                        hcol = slice(half * 512, (half + 1) * 512)
                        hsl = slice(half * 4, (half + 1) * 4)
                        # P = rep(c_t) - c_s + ldt_s  (exact via splits)
                        pw = pwide.tile([128, 512], FP32, tag="pw")
                        nc.tensor.matmul(pw[:], c_ones3[:], crow3[:, hcol],
                                         start=True, stop=False)
                        nc.tensor.matmul(pw[:], cl3[:],
                                         c_sel24n[:, hcol],
                                         start=False, stop=False)
                        nc.tensor.matmul(pw[:], dl2[:],
                                         c_sel16p[:, hcol],
                                         start=False, stop=False)
                        nc.tensor.matmul(pw[:], c_mskl[:],
                                         c_mskr[:, hcol],
                                         start=False, stop=True)
                        dexp = sc1.tile([128, 512], BF16, tag=f"dexp{half}")
                        nc.scalar.activation(dexp[:], pw[:], AF.Exp)
                        dexp3 = dexp[:].rearrange("p (r t) -> p r t", r=4)
                        nc.vector.tensor_mul(mtp3[:, hsl, :], dexp3,
                                             _bc(gm[:], 4))
                        if not first:
                            # erep = exp(rep(c_t)); cec = C * erep
                            er = pwide.tile([128, 512], FP32, tag="pw")
                            nc.tensor.matmul(er[:], c_ones3[:],
                                             crow3[:, hcol],
                                             start=True, stop=True)
                            et = sc1.tile([128, 512], BF16, tag=f"et{half}")
                            nc.scalar.activation(et[:], er[:], AF.Exp)
                            et3 = et[:].rearrange("p (r t) -> p r t", r=4)
                            nc.vector.tensor_mul(cec3[:, hsl, :], et3,
                                                 _bc(xbcc[:, cols], 4))
                            ets.append(et)

                    # token-major X (all heads) and B
                    xtp = ptrp.tile([128, 512], BF16, tag="trp")
                    for pi in range(4):
                        nc.tensor.transpose(
                            xtp[:, pi * Q:(pi + 1) * Q],
                            xall[:, pi * BT + ci * Q:pi * BT + (ci + 1) * Q],
                            c_ibf[:])
                    xtk = sc1.tile([128, 512], BF16, tag="xtk")
                    nc.vector.tensor_copy(xtk[:], xtp[:])
                    btp = ptrp.tile([128, 512], BF16, tag="trp")
                    nc.tensor.transpose(btp[:, 0:Q], xbcb[:, cols], c_ibf[:])
                    btk = sc.tile([128, Q], BF16, tag="btk")
                    nc.vector.tensor_copy(btk[:], btp[:, 0:Q])

                    # xw = X^T * wrow (dt+decay-to-end weights)
                    wr = pwide.tile([128, 512], FP32, tag="pw")
                    nc.tensor.matmul(wr[:], wrowbf[:], c_wsel[:],
                                     start=True, stop=True)
                    xw = sc1.tile([128, 512], BF16, tag="xw")
                    nc.vector.tensor_mul(xw[:], xtk[:], wr[:])

                    # state update: S_new = S_old * exp(c_end) + B^T xw
                    s_sb_new = scst.tile([128, 512], FP32, tag="ssb")
                    s_bf_new = scst.tile([128, 512], BF16, tag="sbf")
                    sp = pys.tile([128, 512], FP32, tag="ys")
                    nc.tensor.matmul(sp[:], btk[:], xw[:], start=True,
                                     stop=True)
                    if first:
                        nc.vector.tensor_copy(s_bf_new[:], sp[:])
                        nc.vector.tensor_copy(s_sb_new[:], sp[:])
                    else:
                        for r in range(NH):
                            esl = slice(r * PD, (r + 1) * PD)
                            dcol = ets[r // 4][:, (r % 4) * Q + Q - 1:
                                               (r % 4) * Q + Q]
                            # bf16 copy first: it gates next chunk's inter
                            nc.vector.scalar_tensor_tensor(
                                s_bf_new[:, esl], s_sb_prev[:, esl], dcol,
                                sp[:, esl], ALU.mult, ALU.add)
                            nc.vector.scalar_tensor_tensor(
                                s_sb_new[:, esl], s_sb_prev[:, esl], dcol,
                                sp[:, esl], ALU.mult, ALU.add)

                    # Y = X M + S_prev cec + D x   (per head pair, one PSUM)
                    yp = pys.tile([128, 512], FP32, tag="ys")
                    for pi in range(4):
                        pcol = slice(pi * Q, (pi + 1) * Q)
                        for hh in range(2):
                            r = pi * 2 + hh
                            orow = slice(hh * PD, (hh + 1) * PD)
                            nc.tensor.matmul(
                                yp[orow, pcol],
                                xtk[:, r * PD:(r + 1) * PD],
                                mtp[:, r * Q:(r + 1) * Q],
                                start=True, stop=False)
                            if not first:
                                nc.tensor.matmul(
                                    yp[orow, pcol],
                                    s_bf_prev[:, r * PD:(r + 1) * PD],
                                    cec[:, r * Q:(r + 1) * Q],
                                    start=False, stop=False)
                        nc.tensor.matmul(
                            yp[:, pcol], c_diag4[:, pi * 128:(pi + 1) * 128],
                            xall[:, pi * BT + ci * Q:pi * BT + (ci + 1) * Q],
                            start=False, stop=True)

                    # v = (Y + D x) * silu(z), written over x in place
                    yp3 = yp[:].rearrange("p (a t) -> p a t", a=4)
                    nc.vector.tensor_mul(xav[:, :, cols], yp3,
                                         szv[:, :, cols])

                    if _DBG and ci == 0:
                        nc.sync.dma_start(dbg_mtp[:], mtp[:])
                        nc.sync.dma_start(dbg_ssb[:], s_sb_new[:])
                        nc.sync.dma_start(dbg_xtk[:], xtk[:])
                    if _DBG and ci == 1:
                        nc.sync.dma_start(dbg_cec[:], cec[:])
                    if _DBG and ci == NCH - 1:
                        nc.sync.dma_start(dbg_v[:], xall[:])

                    s_sb_prev, s_bf_prev = s_sb_new, s_bf_new

                    # ---- out_proj + sumsq + RS + final for finished group --
                    if ci % 4 != 3:
                        continue
                    g = ci // 4
                    gc = slice(g * GSZ, (g + 1) * GSZ)
                    v2 = [op.tile([128, GSZ], BF16, tag=f"v2_{e % 2}",
                                  name=f"v2_{g}_{e}")
                          for e in range(4)]
                    ssp = pys.tile([1, GSZ], FP32, tag="ys")
                    for e in range(4):
                        nc.scalar.activation(
                            v2[e][:], xall[:, e * BT + g * GSZ:
                                           e * BT + (g + 1) * GSZ],
                            AF.Square)
                        nc.tensor.matmul(ssp[:], c_ones_col[:], v2[e][:],
                                         start=(e == 0), stop=(e == 3))
                    ssq = op.tile([1, GSZ], BF16, tag="ssq")
                    nc.scalar.copy(ssq[:], ssp[:])

                    for tt in range(4):
                        trows = slice(g * GSZ + tt * 128,
                                      g * GSZ + (tt + 1) * 128)
                        for oh in range(2):
                            hw = 1024 if oh == 0 else RSW - 1024
                            osb = op.tile([128, hw], BF16, tag="osb",
                                          name=f"osb{g}_{tt}_{oh}")
                            for n in (0, 1):
                                n2 = oh * 2 + n
                                ncol = slice(n * 512, (n + 1) * 512)
                                outp = pmm.tile([128, 512], FP32, tag="mm")
                                for k in range(4):
                                    nc.tensor.matmul(
                                        outp[:],
                                        xall[:,
                                             k * BT + g * GSZ + tt * 128:
                                             k * BT + g * GSZ + (tt + 1) * 128],
                                        wo[k][:, n2 * 512:(n2 + 1) * 512],
                                        start=(k == 0), stop=(k == 3))
                                nc.vector.tensor_copy(osb[:, ncol], outp[:])
                            if oh == 1:
                                sqp = ptrp.tile([128, 512], BF16, tag="trp")
                                nc.tensor.transpose(
                                    sqp[0:128, 0:1],
                                    ssq[0:1, tt * 128:(tt + 1) * 128],
                                    c_ibf[0:1, 0:1])
                                nc.vector.tensor_copy(
                                    osb[:, 1024:1025], sqp[:, 0:1])
                                nc.vector.memset(osb[:, 1025:hw], 0.0)
                            nc.sync.dma_start(
                                rs_in[g][tt * 128:(tt + 1) * 128,
                                         oh * 1024:oh * 1024 + hw], osb[:])
                    if _STOP == "nocc":
                        continue
                    nc.gpsimd.collective_compute(
                        "ReduceScatter", ALU.add, replica_groups=rg,
                        ins=[rs_in[g].opt()], outs=[rs_out[g].opt()])

                    # final gated-norm scale on own token shard
                    gsb = fin.tile([64, 1], BF16, tag="gsb")
                    nc.sync.dma_start(gsb[:], rs_out[g][:, 2048:2049])
                    gln = fin.tile([64, 1], FP32, tag="gln")
                    nc.scalar.activation(gln[:], gsb[:], AF.Ln,
                                         scale=1.0 / (2 * DM),
                                         bias=c_eps[0:64, :])
                    gcol = fin.tile([64, 1], FP32, tag="gcol")
                    nc.scalar.activation(gcol[:], gln[:], AF.Exp, scale=-0.5)
                    for hf in range(8):
                        cl = slice(hf * (DM // 8), (hf + 1) * (DM // 8))
                        ld = fin.tile([64, DM // 8], BF16, tag="ld",
                                      name=f"ld{g}_{hf}")
                        nc.sync.dma_start(ld[:], rs_out[g][:, cl])
                        fo = fin.tile([64, DM // 8], BF16, tag="fo",
                                      name=f"fo{g}_{hf}")
                        nc.vector.tensor_scalar_mul(fo[:], ld[:], gcol[:])
                        nc.sync.dma_start(out_rs[g * 64:(g + 1) * 64, cl],
                                          fo[:])

        except _StopBuild:
            pass
    nc.compile()
    return nc


def _get_built():
    global _BUILT
    if _BUILT is None:
        _BUILT = _build()
    return _BUILT


def kernel(**inputs):
    hs = np.ascontiguousarray(np.asarray(inputs["hidden_states"],
                                         dtype=np.float32))
    rd = np.ascontiguousarray(np.asarray(inputs["residual"], dtype=np.float32))
    B, L, Dm = hs.shape
    norm_w = np.asarray(inputs["norm_w"], dtype=np.float32)
    in_w = np.asarray(inputs["in_proj_w"], dtype=np.float32)
    conv_w = np.asarray(inputs["conv_w"], dtype=np.float32)
    conv_b = np.asarray(inputs["conv_b"], dtype=np.float32)
    A_log = np.asarray(inputs["A_log"], dtype=np.float32)
    D_param = np.asarray(inputs["D_param"], dtype=np.float32)
    dt_bias = np.asarray(inputs["dt_bias"], dtype=np.float32)
    gnw = np.asarray(inputs["gate_norm_w"], dtype=np.float32)
    out_w = np.asarray(inputs["out_proj_w"], dtype=np.float32)

    hid2 = hs.reshape(BT, DM)
    res2 = rd.reshape(BT, DM)
    Wn = in_w * norm_w[None, :]
    Wg = out_w * gnw[None, :]

    # select matrices for the wide decay matmul
    sel24n = np.zeros((24, NH * Q), np.float32)
    sel16p = np.zeros((16, NH * Q), np.float32)
    for lvl in range(3):
        for r in range(NH):
            sel24n[8 * lvl + r, r * Q:(r + 1) * Q] = -1.0
    for lvl in range(2):
        for r in range(NH):
            sel16p[8 * lvl + r, r * Q:(r + 1) * Q] = 1.0
    # causal mask as a rank-128 matmul term: msk_l^T @ msk_r = -1e30*[s>t]
    msk_l = (np.arange(Q)[None, :] > np.arange(Q)[:, None]) \
        .astype(np.float32)                            # [k, s] = [s > k]
    msk_r1 = np.where(np.arange(Q)[:, None] == np.arange(Q)[None, :],
                      np.float32(-1e30), np.float32(0.0))  # [k, t] = -1e30*[t==k]
    msk_r = np.tile(msk_r1, (1, NH))
    wselm = np.zeros((NH, DI), np.float32)
    for r in range(NH):
        wselm[r, r * PD:(r + 1) * PD] = 1.0

    common = {
        "hid": hid2, "res": res2,
        "ones_col_bf": np.ones((128, 1), ml_dtypes.bfloat16),
        "i_bf": np.eye(128, dtype=ml_dtypes.bfloat16),
        "ones3": np.ones((3, 128), ml_dtypes.bfloat16),
        "sel24n": sel24n.astype(ml_dtypes.bfloat16),
        "sel16p": sel16p.astype(ml_dtypes.bfloat16),
        "msk_l": msk_l.astype(ml_dtypes.bfloat16),
        "msk_r": msk_r.astype(ml_dtypes.bfloat16),
        "wsel": wselm.astype(ml_dtypes.bfloat16),
    }

    in_maps = []
    for c in range(NCORES):
        rows = np.r_[4096 + 512 * c:4096 + 512 * (c + 1),
                     8192 + 128 * c:8192 + 128 * (c + 1),
                     9216 + 128 * c:9216 + 128 * (c + 1),
                     512 * c:512 * (c + 1),
                     10240 + 8 * c:10240 + 8 * (c + 1)]
        w_in_t = np.ascontiguousarray(Wn[rows, :].T).astype(ml_dtypes.bfloat16)
        w_out_t = np.ascontiguousarray(
            Wg[:, 512 * c:512 * (c + 1)].T).astype(ml_dtypes.bfloat16)
        crows = np.r_[512 * c:512 * (c + 1),
                      4096 + 128 * c:4096 + 128 * (c + 1),
                      5120 + 128 * c:5120 + 128 * (c + 1)]
        diag4 = np.zeros((128, 512), np.float32)
        for pi in range(4):
            dpair = np.repeat(D_param[8 * c + 2 * pi:8 * c + 2 * pi + 2], PD)
            diag4[:, pi * 128:(pi + 1) * 128] = np.diag(dpair)
        in_maps.append(dict(
            common,
            w_in_t=w_in_t,
            w_out_t=w_out_t,
            a_col=(-np.exp(A_log[8 * c:8 * (c + 1)])).reshape(8, 1)
                  .astype(np.float32),
            dtb_col=dt_bias[8 * c:8 * (c + 1)].reshape(8, 1).astype(np.float32),
            diag4=diag4.astype(ml_dtypes.bfloat16),
            convw=np.ascontiguousarray(
                conv_w[crows, :].reshape(6, 128, 4).transpose(1, 0, 2)
                .reshape(128, 24)).astype(np.float32),
            convb=np.ascontiguousarray(
                conv_b[crows].reshape(6, 128).T).astype(np.float32),
        ))

    nc = _get_built()
    res_k = bass_utils.run_bass_kernel_spmd(
        nc, in_maps, core_ids=list(range(NCORES)))
    global LAST_RESULTS
    LAST_RESULTS = res_k

    out = np.empty((BT, DM), np.float32)
    for c in range(NCORES):
        o = np.asarray(res_k.results[c]["out_rs"]).astype(np.float32)
        for g in range(NGRP):
            out[g * GSZ + c * 64:g * GSZ + (c + 1) * 64, :] = \
                o[g * 64:(g + 1) * 64, :]
    new_res = np.asarray(res_k.results[0]["new_res"]).astype(np.float32)
    return out.reshape(B, L, Dm), new_res.reshape(B, L, Dm)


# revision 69
# speedup vs baseline: 1.0018x; 1.0018x over previous
"""NemotronH Mamba2 decoder layer on 8 Trainium2 cores (Bass/Tile).

Sharding: tensor-parallel over the 8 SSM groups (1 group = 8 heads / core).
in_proj, conv, A, D, dt_bias sharded along d_inner; out_proj sharded along its
input dim with a pipelined bf16 ReduceScatter (gate-norm sumsq folded in as an
extra RS column); RMSNorm replicated.

Pipeline per 512-token group g: pass A -> in_proj -> conv -> dt prep; scan
chunks 4g..4g+3 overlap in_proj of g+1; out_proj + RS + final scale for group
g are emitted right after scan chunk 4g+3 so the collectives overlap the scan.

Scan (chunked SSD, Q=128) uses wide ops: the per-head decay matrix exponent
c_t - c_s + ldt_s is built for all 8 heads in one PSUM tile via select-matmuls
(3-level bf16 hi/lo/lolo splits keep it exact); the causal mask is an additive
-1e30 const; G = B^T C is shared via stride-0 broadcast APs; D is folded in via
diagonal matmuls into the Y PSUM; v = (Y + Dx) * silu(z) is one strided mul.
"""
import contextlib
import os
import sys
import types

import numpy as np
import ml_dtypes

# --- axon NTFF profile hook shim (lets trace=True work in this container) ---
try:
    import antenv
    if "antenv.axon_hooks" not in sys.modules:
        try:
            from trn_agent_boot.trn_boot import _ntff_profile_via_ctypes
            _hooks = types.ModuleType("antenv.axon_hooks")
            _hook = _ntff_profile_via_ctypes("/opt/axon/libaxon_pjrt.so")
            _hooks.get_axon_ntff_profile_hook = lambda: _hook
            sys.modules["antenv.axon_hooks"] = _hooks
            antenv.axon_hooks = _hooks
        except Exception:
            pass
except Exception:
    pass

import concourse.bass as bass
import concourse.bacc as bacc
import concourse.tile as tile
import concourse.mybir as mybir
import concourse.bass_utils as bass_utils

bass_utils.upload_artifacts = lambda tmpdir: tmpdir  # no S3 in-container

FP32 = mybir.dt.float32
BF16 = mybir.dt.bfloat16
AF = mybir.ActivationFunctionType
ALU = mybir.AluOpType

NCORES = 8
BT = 2048        # B*L tokens
DM = 2048        # model dim
DI = 512         # d_inner slice per core (8 heads x 64)
NH = 8           # heads per core
PD = 64          # head dim
Q = 128          # scan chunk length
NCH = BT // Q    # 16 chunks
NGRP = 4         # token groups for pipelining
GSZ = BT // NGRP # 512
EPS = 1e-5
CVC = 518        # conv buffer cols: 3 history + 512 + 3 slack
RSW = 2056       # RS row width: 2048 out + 1 sumsq + 7 pad

_BUILT = None
LAST_RESULTS = None


def _bc(ap, n):
    """Insert a stride-0 middle free dim of size n: [P, W] -> [P, n, W]."""
    return bass.AP(ap.tensor, ap.offset, [ap.ap[0], [0, n]] + ap.ap[1:])


class _StopBuild(Exception):
    pass
_STOP = os.environ.get("K_STOP", "full")  # passa|inproj|nocc|full
_DBG = os.environ.get("K_DBG") == "1"


def _build():
    nc = bacc.Bacc("TRN2", target_bir_lowering=False, debug=False,
                   num_devices=NCORES)

    def inp(name, shape, dt):
        return nc.dram_tensor(name, shape, dt, kind="ExternalInput").ap()

    hid = inp("hid", [BT, DM], FP32)
    res = inp("res", [BT, DM], FP32)
    w_in_t = inp("w_in_t", [DM, 1288], BF16)
    w_out_t = inp("w_out_t", [DI, DM], BF16)
    a_col = inp("a_col", [NH, 1], FP32)
    dtb_col = inp("dtb_col", [NH, 1], FP32)
    convw = inp("convw", [128, 24], FP32)
    convb = inp("convb", [128, 6], FP32)
    ones_col_bf = inp("ones_col_bf", [128, 1], BF16)
    i_bf = inp("i_bf", [128, 128], BF16)
    ones3 = inp("ones3", [3, 128], BF16)
    sel24n = inp("sel24n", [24, NH * Q], BF16)
    sel16p = inp("sel16p", [16, NH * Q], BF16)
    m0t8 = inp("m0t8", [128, NH * Q], BF16)
    wsel = inp("wsel", [NH, DI], BF16)
    diag4 = inp("diag4", [128, 512], BF16)

    new_res = nc.dram_tensor("new_res", [BT, DM], BF16,
                             kind="ExternalOutput").ap()
    out_rs = nc.dram_tensor("out_rs", [256, DM], BF16,
                            kind="ExternalOutput").ap()

    rg = [list(range(NCORES))]

    if _DBG:
        dbg_mtp = nc.dram_tensor("dbg_mtp", [128, NH * Q], BF16,
                                 kind="ExternalOutput").ap()
        dbg_cec = nc.dram_tensor("dbg_cec", [128, NH * Q], BF16,
                                 kind="ExternalOutput").ap()
        dbg_ssb = nc.dram_tensor("dbg_ssb", [128, 512], FP32,
                                 kind="ExternalOutput").ap()
        dbg_xtk = nc.dram_tensor("dbg_xtk", [128, 512], BF16,
                                 kind="ExternalOutput").ap()
        dbg_v = nc.dram_tensor("dbg_v", [128, 4 * BT], BF16,
                               kind="ExternalOutput").ap()

    with tile.TileContext(nc) as tc:
        try:
            with contextlib.ExitStack() as stack:
                ec = stack.enter_context
                cpool = ec(tc.tile_pool(name="const", bufs=1))
                dram = ec(tc.tile_pool(name="dram", bufs=1, space="DRAM"))
                mid = ec(tc.tile_pool(name="mid", bufs=1))
                wpool = ec(tc.tile_pool(name="wpool", bufs=1))
                convp = ec(tc.tile_pool(name="convp", bufs=1))
                pa = ec(tc.tile_pool(name="pa", bufs=2))
                ip = ec(tc.tile_pool(name="ip", bufs=2))
                ipx = ec(tc.tile_pool(name="ipx", bufs=1))
                sc = ec(tc.tile_pool(name="sc", bufs=2))
                sc1 = ec(tc.tile_pool(name="sc1", bufs=1))
                scst = ec(tc.tile_pool(name="scst", bufs=2))
                op = ec(tc.tile_pool(name="op", bufs=1))
                fin = ec(tc.tile_pool(name="fin", bufs=1))
                pmm = ec(tc.tile_pool(name="pmm", bufs=2, space="PSUM"))
                pwide = ec(tc.tile_pool(name="pwide", bufs=2, space="PSUM"))
                ptrp = ec(tc.tile_pool(name="ptrp", bufs=2, space="PSUM"))
                pys = ec(tc.tile_pool(name="pys", bufs=2, space="PSUM"))

                # ---------------- constants ----------------
                c_ones_col = cpool.tile([128, 1], BF16)
                nc.sync.dma_start(c_ones_col[:], ones_col_bf[:])
                c_ibf = cpool.tile([128, 128], BF16)
                nc.sync.dma_start(c_ibf[:], i_bf[:])
                c_acol = cpool.tile([NH, 1], FP32)
                nc.sync.dma_start(c_acol[:], a_col[:])
                c_dtb = cpool.tile([NH, 1], FP32)
                nc.sync.dma_start(c_dtb[:], dtb_col[:])
                c_convw = cpool.tile([128, 24], FP32)
                nc.sync.dma_start(c_convw[:], convw[:])
                c_convb = cpool.tile([128, 6], FP32)
                nc.sync.dma_start(c_convb[:], convb[:])
                c_ones3 = cpool.tile([3, 128], BF16)
                nc.sync.dma_start(c_ones3[:], ones3[:])
                c_sel24n = cpool.tile([24, NH * Q], BF16)
                nc.sync.dma_start(c_sel24n[:], sel24n[:])
                c_sel16p = cpool.tile([16, NH * Q], BF16)
                nc.sync.dma_start(c_sel16p[:], sel16p[:])
                c_m0t8 = cpool.tile([128, NH * Q], BF16)
                nc.sync.dma_start(c_m0t8[:], m0t8[:])
                c_wsel = cpool.tile([NH, DI], BF16)
                nc.sync.dma_start(c_wsel[:], wsel[:])
                c_diag4 = cpool.tile([128, 512], BF16)
                nc.sync.dma_start(c_diag4[:], diag4[:])
                z8 = cpool.tile([NH, Q], FP32)
                nc.vector.memset(z8[:], 0.0)
                c_eps = cpool.tile([128, 1], FP32)
                nc.vector.memset(c_eps[:], EPS)

                xs_dram = dram.tile([BT, DM], BF16)
                rs_in = [dram.tile([GSZ, RSW], BF16, name=f"rsin{g}")
                         for g in range(NGRP)]
                rs_out = [dram.tile([64, RSW], BF16, name=f"rsout{g}")
                          for g in range(NGRP)]

                # ---------------- weight prefetch ----------------
                wt = [wpool.tile([128, 1288], BF16, name=f"wt{k}")
                      for k in range(16)]
                for k in range(16):
                    nc.sync.dma_start(wt[k][:], w_in_t[k * 128:(k + 1) * 128, :])
                wo = [wpool.tile([128, DM], BF16, name=f"wo{k}")
                      for k in range(4)]
                for k in range(4):
                    nc.sync.dma_start(wo[k][:], w_out_t[k * 128:(k + 1) * 128, :])

                # ---------------- mid-life activations ----------------
                xall = mid.tile([128, 4 * BT], BF16)     # x, then v in place
                szall = mid.tile([128, 4 * BT], BF16)    # silu(z)
                xbcb = mid.tile([128, BT], BF16)         # B (feat-major)
                xbcc = mid.tile([128, BT], BF16)         # C (feat-major)
                ldt = mid.tile([NH, BT], FP32)
                a_row = mid.tile([NH, BT], FP32)
                rsq = mid.tile([128, 16], FP32)

                cvb = [convp.tile([128, CVC], BF16, name=f"cvb{i}")
                       for i in range(6)]
                for i in range(6):
                    nc.vector.memset(cvb[i][:, 0:3], 0.0)

                xav = xall[:].rearrange("p (a t) -> p a t", a=4)
                szv = szall[:].rearrange("p (a t) -> p a t", a=4)

                # ============ pipelined groups: passA -> in_proj -> conv ====
                for g in range(NGRP):
                    gc = slice(g * GSZ, (g + 1) * GSZ)
                    # ---- pass A (fused): h, new_res, rsqrt, xs ----
                    HW = DM // 2
                    for i in range(4):
                        ti = 4 * g + i
                        rows = slice(ti * 128, (ti + 1) * 128)
                        hts, sss = [], []
                        for hf in range(2):
                            cl = slice(hf * HW, (hf + 1) * HW)
                            th = pa.tile([128, HW], FP32, tag="hid")
                            tr = pa.tile([128, HW], FP32, tag="res")
                            nc.sync.dma_start(th[:], hid[rows, cl])
                            nc.sync.dma_start(tr[:], res[rows, cl])
                            hsum = pa.tile([128, HW], BF16, tag=f"h{hf}")
                            nc.vector.tensor_add(hsum[:], th[:], tr[:])
                            nc.sync.dma_start(new_res[rows, cl], hsum[:])
                            sq = pa.tile([128, HW], BF16, tag="xs")
                            ss = pa.tile([128, 1], FP32, tag=f"ss{hf}")
                            nc.scalar.activation(sq[:], hsum[:], AF.Square,
                                                 accum_out=ss[:])
                            hts.append(hsum)
                            sss.append(ss)
                        ssf = pa.tile([128, 1], FP32, tag="ssf")
                        nc.vector.tensor_add(ssf[:], sss[0][:], sss[1][:])
                        ln = pa.tile([128, 1], FP32, tag="ln")
                        nc.scalar.activation(ln[:], ssf[:], AF.Ln,
                                             scale=1.0 / DM, bias=c_eps[:])
                        nc.scalar.activation(rsq[:, ti:ti + 1], ln[:], AF.Exp,
                                             scale=-0.5)
                        for hf in range(2):
                            cl = slice(hf * HW, (hf + 1) * HW)
                            xsb = pa.tile([128, HW], BF16, tag="xs")
                            nc.vector.tensor_scalar_mul(xsb[:], hts[hf][:],
                                                        rsq[:, ti:ti + 1])
                            nc.sync.dma_start(xs_dram[rows, cl], xsb[:])

                    # ---- x^T k-tiles for this group ----
                    xt = [ipx.tile([128, GSZ], BF16, tag=f"xt{k}",
                                   name=f"xt{g}_{k}") for k in range(16)]
                    for k in range(16):
                        nc.sync.dma_start_transpose(
                            xt[k][:], xs_dram[gc, k * 128:(k + 1) * 128])

                    if _STOP == "passa":
                        continue
                    # ---- in_proj: M-tiles 0-3 x, 4 B, 5 C, 6-9 z, 10 dt ----
                    dt_raw = ip.tile([NH, GSZ], FP32, tag="dtraw")
                    for m in range(11):
                        mrows = 8 if m == 10 else 128
                        ps = pmm.tile([128, GSZ], FP32, tag="mm")
                        for k in range(16):
                            nc.tensor.matmul(
                                ps[0:mrows, :],
                                wt[k][:, m * 128:m * 128 + mrows],
                                xt[k][:],
                                start=(k == 0), stop=(k == 15))
                        if m < 6:
                            nc.scalar.copy(cvb[m][:, 3:3 + GSZ], ps[:, :])
                        elif m < 10:
                            nc.scalar.activation(
                                szall[:, (m - 6) * BT + g * GSZ:
                                      (m - 6) * BT + (g + 1) * GSZ],
                                ps[:, :], AF.Silu)
                        else:
                            nc.scalar.copy(dt_raw[:], ps[0:8, :])

                    # ---- conv + silu (bf16 chain) ----
                    for i in range(6):
                        cw = [c_convw[:, i * 4 + k:i * 4 + k + 1]
                              for k in range(4)]
                        t0 = ip.tile([128, GSZ], BF16, tag="cv")
                        nc.vector.tensor_scalar_mul(t0[:], cvb[i][:, 0:GSZ],
                                                    cw[0])
                        t1 = ip.tile([128, GSZ], BF16, tag="cv")
                        nc.vector.scalar_tensor_tensor(
                            t1[:], cvb[i][:, 1:1 + GSZ], cw[1], t0[:],
                            ALU.mult, ALU.add)
                        t2 = ip.tile([128, GSZ], BF16, tag="cv")
                        nc.vector.scalar_tensor_tensor(
                            t2[:], cvb[i][:, 2:2 + GSZ], cw[2], t1[:],
                            ALU.mult, ALU.add)
                        t3 = ip.tile([128, GSZ], BF16, tag="cv")
                        nc.vector.scalar_tensor_tensor(
                            t3[:], cvb[i][:, 3:3 + GSZ], cw[3], t2[:],
                            ALU.mult, ALU.add)
                        if i < 4:
                            dst = xall[:, i * BT + g * GSZ:
                                       i * BT + (g + 1) * GSZ]
                        elif i == 4:
                            dst = xbcb[:, gc]
                        else:
                            dst = xbcc[:, gc]
                        nc.scalar.activation(dst, t3[:], AF.Silu,
                                             bias=c_convb[:, i:i + 1])
                        # roll conv history (zero across the batch boundary)
                        if g == 1:
                            nc.vector.memset(cvb[i][:, 0:3], 0.0)
                        else:
                            nc.vector.tensor_copy(cvb[i][:, 0:3],
                                                  cvb[i][:, GSZ:GSZ + 3])

                    # ---- dt prep for this group ----
                    e1 = ip.tile([NH, GSZ], FP32, tag="dtraw")
                    nc.scalar.activation(e1[:], dt_raw[:], AF.Exp,
                                         bias=c_dtb[:])
                    e2 = ip.tile([NH, GSZ], FP32, tag="dtraw")
                    nc.vector.tensor_scalar_add(e2[:], e1[:], 1.0)
                    dt_v = ip.tile([NH, GSZ], FP32, tag="dtraw")
                    nc.scalar.activation(dt_v[:], e2[:], AF.Ln)
                    nc.scalar.activation(ldt[:, gc], dt_v[:], AF.Ln)
                    nc.vector.tensor_scalar_mul(a_row[:, gc], dt_v[:],
                                                c_acol[:])

                # ---------------- scan + interleaved out_proj/RS ------------
                if _STOP in ("passa", "inproj"):
                    raise _StopBuild()
                s_bf_prev = None
                s_sb_prev = None
                for ci in range(NCH):
                    cols = slice(ci * Q, (ci + 1) * Q)
                    first = (ci % 8 == 0)

                    # cumsum of A*dt, wrow = exp(ldt - c + c_end)
                    c_t = sc.tile([NH, Q], FP32, tag="c")
                    nc.vector.tensor_tensor_scan(
                        c_t[:], a_row[:, cols], z8[:], 0.0, ALU.add, ALU.add)
                    lc = sc.tile([NH, Q], FP32, tag="lc")
                    nc.vector.tensor_sub(lc[:], ldt[:, cols], c_t[:])
                    wrow = sc.tile([NH, Q], FP32, tag="lc")
                    nc.scalar.activation(wrow[:], lc[:], AF.Exp,
                                         bias=c_t[:, Q - 1:Q])
                    wrowbf = sc.tile([NH, Q], BF16, tag="btk")
                    nc.vector.tensor_copy(wrowbf[:], wrow[:])

                    # 3-level split of c + 2-level split of ldt (exact in sum)
                    # (compute engines can't write partition offsets that are
                    #  not 32-aligned -> stage split rows via SBUF-SBUF DMA)
                    cl3 = sc.tile([24, Q], BF16, tag="cl3")
                    dl2 = sc.tile([16, Q], BF16, tag="cl3")
                    r1 = sc.tile([NH, Q], FP32, tag="lc")
                    r2 = sc.tile([NH, Q], FP32, tag="lc")
                    clo = sc1.tile([NH, Q], BF16, tag="clo")
                    cll = sc1.tile([NH, Q], BF16, tag="cll")
                    nc.vector.tensor_copy(cl3[0:8, :], c_t[:])
                    nc.vector.tensor_sub(r1[:], c_t[:], cl3[0:8, :])
                    nc.vector.tensor_copy(clo[:], r1[:])
                    nc.vector.tensor_sub(r2[:], r1[:], clo[:])
                    nc.vector.tensor_copy(cll[:], r2[:])
                    nc.sync.dma_start(cl3[8:16, :], clo[:])
                    nc.sync.dma_start(cl3[16:24, :], cll[:])
                    nc.vector.tensor_copy(dl2[0:8, :], ldt[:, cols])
                    r3 = sc1.tile([NH, Q], FP32, tag="r1")
                    nc.vector.tensor_sub(r3[:], ldt[:, cols], dl2[0:8, :])
                    dlo = sc1.tile([NH, Q], BF16, tag="clo")
                    nc.vector.tensor_copy(dlo[:], r3[:])
                    nc.sync.dma_start(dl2[8:16, :], dlo[:])

                    crow3 = sc1.tile([3, NH * Q], BF16, tag="crow3")
                    nc.sync.dma_start(crow3[0:1, :], cl3[0:8, :])
                    nc.sync.dma_start(crow3[1:2, :], clo[:])
                    nc.sync.dma_start(crow3[2:3, :], cll[:])

                    # G = B^T C (shared across heads)
                    gmp = pwide.tile([128, 512], FP32, tag="pw")
                    nc.tensor.matmul(gmp[:, 0:Q], xbcb[:, cols],
                                     xbcc[:, cols], start=True, stop=True)
                    gm = sc1.tile([128, Q], BF16, tag="gm")
                    nc.vector.tensor_copy(gm[:], gmp[:, 0:Q])

                    mtp = sc.tile([128, NH * Q], BF16, tag="mtp")
                    mtp3 = mtp[:].rearrange("p (r t) -> p r t", r=NH)
                    cec = sc1.tile([128, NH * Q], BF16, tag="cec")
                    cec3 = cec[:].rearrange("p (r t) -> p r t", r=NH)
                    ets = []
                    for half in range(2):
                        hcol = slice(half * 512, (half + 1) * 512)
                        hsl = slice(half * 4, (half + 1) * 4)
                        # P = rep(c_t) - c_s + ldt_s  (exact via splits)
                        pw = pwide.tile([128, 512], FP32, tag="pw")
                        nc.tensor.matmul(pw[:], c_ones3[:], crow3[:, hcol],
                                         start=True, stop=False)
                        nc.tensor.matmul(pw[:], cl3[:],
                                         c_sel24n[:, hcol],
                                         start=False, stop=False)
                        nc.tensor.matmul(pw[:], dl2[:],
                                         c_sel16p[:, hcol],
                                         start=False, stop=True)
                        nc.vector.tensor_add(pw[:], pw[:], c_m0t8[:, hcol])
                        dexp = sc1.tile([128, 512], BF16, tag="dexp",
                                        name=f"dexp{ci}_{half}")
                        nc.scalar.activation(dexp[:], pw[:], AF.Exp)
                        dexp3 = dexp[:].rearrange("p (r t) -> p r t", r=4)
                        nc.vector.tensor_mul(mtp3[:, hsl, :], dexp3,
                                             _bc(gm[:], 4))
                        if not first:
                            # erep = exp(rep(c_t)); cec = C * erep
                            er = pwide.tile([128, 512], FP32, tag="pw")
                            nc.tensor.matmul(er[:], c_ones3[:],
                                             crow3[:, hcol],
                                             start=True, stop=True)
                            et = sc1.tile([128, 512], BF16, tag=f"et{half}")
                            nc.scalar.activation(et[:], er[:], AF.Exp)
                            et3 = et[:].rearrange("p (r t) -> p r t", r=4)
                            nc.vector.tensor_mul(cec3[:, hsl, :], et3,
                                                 _bc(xbcc[:, cols], 4))
                            ets.append(et)

                    # token-major X (all heads) and B
                    xtp = ptrp.tile([128, 512], BF16, tag="trp")
                    for pi in range(4):
                        nc.tensor.transpose(
                            xtp[:, pi * Q:(pi + 1) * Q],
                            xall[:, pi * BT + ci * Q:pi * BT + (ci + 1) * Q],
                            c_ibf[:])
                    xtk = sc.tile([128, 512], BF16, tag="xtk")
                    nc.vector.tensor_copy(xtk[:], xtp[:])
                    btp = ptrp.tile([128, 512], BF16, tag="trp")
                    nc.tensor.transpose(btp[:, 0:Q], xbcb[:, cols], c_ibf[:])
                    btk = sc.tile([128, Q], BF16, tag="btk")
                    nc.vector.tensor_copy(btk[:], btp[:, 0:Q])

                    # xw = X^T * wrow (dt+decay-to-end weights)
                    wr = pwide.tile([128, 512], FP32, tag="pw")
                    nc.tensor.matmul(wr[:], wrowbf[:], c_wsel[:],
                                     start=True, stop=True)
                    xw = sc1.tile([128, 512], BF16, tag="xw")
                    nc.vector.tensor_mul(xw[:], xtk[:], wr[:])

                    # state update: S_new = S_old * exp(c_end) + B^T xw
                    s_sb_new = scst.tile([128, 512], FP32, tag="ssb")
                    s_bf_new = scst.tile([128, 512], BF16, tag="sbf")
                    sp = pys.tile([128, 512], FP32, tag="ys")
                    nc.tensor.matmul(sp[:], btk[:], xw[:], start=True,
                                     stop=True)
                    if first:
                        nc.vector.tensor_copy(s_sb_new[:], sp[:])
                    else:
                        for r in range(NH):
                            esl = slice(r * PD, (r + 1) * PD)
                            dcol = ets[r // 4][:, (r % 4) * Q + Q - 1:
                                               (r % 4) * Q + Q]
                            nc.vector.scalar_tensor_tensor(
                                s_sb_new[:, esl], s_sb_prev[:, esl], dcol,
                                sp[:, esl], ALU.mult, ALU.add)
                    nc.vector.tensor_copy(s_bf_new[:], s_sb_new[:])

                    # Y = X M + S_prev cec + D x   (per head pair, one PSUM)
                    yp = pys.tile([128, 512], FP32, tag="ys")
                    for pi in range(4):
                        pcol = slice(pi * Q, (pi + 1) * Q)
                        for hh in range(2):
                            r = pi * 2 + hh
                            orow = slice(hh * PD, (hh + 1) * PD)
                            nc.tensor.matmul(
                                yp[orow, pcol],
                                xtk[:, r * PD:(r + 1) * PD],
                                mtp[:, r * Q:(r + 1) * Q],
                                start=True, stop=False)
                            if not first:
                                nc.tensor.matmul(
                                    yp[orow, pcol],
                                    s_bf_prev[:, r * PD:(r + 1) * PD],
                                    cec[:, r * Q:(r + 1) * Q],
                                    start=False, stop=False)
                        nc.tensor.matmul(
                            yp[:, pcol], c_diag4[:, pi * 128:(pi + 1) * 128],
                            xall[:, pi * BT + ci * Q:pi * BT + (ci + 1) * Q],
                            start=False, stop=True)

                    # v = (Y + D x) * silu(z), written over x in place
                    yp3 = yp[:].rearrange("p (a t) -> p a t", a=4)
                    nc.vector.tensor_mul(xav[:, :, cols], yp3,
                                         szv[:, :, cols])

                    if _DBG and ci == 0:
                        nc.sync.dma_start(dbg_mtp[:], mtp[:])
                        nc.sync.dma_start(dbg_ssb[:], s_sb_new[:])
                        nc.sync.dma_start(dbg_xtk[:], xtk[:])
                    if _DBG and ci == 1:
                        nc.sync.dma_start(dbg_cec[:], cec[:])
                    if _DBG and ci == NCH - 1:
                        nc.sync.dma_start(dbg_v[:], xall[:])

                    s_sb_prev, s_bf_prev = s_sb_new, s_bf_new

                    # ---- out_proj + sumsq + RS + final for finished group --
                    if ci % 4 != 3:
                        continue
                    g = ci // 4
                    gc = slice(g * GSZ, (g + 1) * GSZ)
                    v2 = [op.tile([128, GSZ], BF16, tag="v2",
                                  name=f"v2_{g}_{e}")
                          for e in range(4)]
                    ssp = pys.tile([1, GSZ], FP32, tag="ys")
                    for e in range(4):
                        nc.scalar.activation(
                            v2[e][:], xall[:, e * BT + g * GSZ:
                                           e * BT + (g + 1) * GSZ],
                            AF.Square)
                        nc.tensor.matmul(ssp[:], c_ones_col[:], v2[e][:],
                                         start=(e == 0), stop=(e == 3))
                    ssq = op.tile([1, GSZ], BF16, tag="ssq")
                    nc.scalar.copy(ssq[:], ssp[:])

                    for tt in range(4):
                        trows = slice(g * GSZ + tt * 128,
                                      g * GSZ + (tt + 1) * 128)
                        for oh in range(2):
                            hw = 1024 if oh == 0 else RSW - 1024
                            osb = op.tile([128, hw], BF16, tag="osb",
                                          name=f"osb{g}_{tt}_{oh}")
                            for n in (0, 1):
                                n2 = oh * 2 + n
                                ncol = slice(n * 512, (n + 1) * 512)
                                outp = pmm.tile([128, 512], FP32, tag="mm")
                                for k in range(4):
                                    nc.tensor.matmul(
                                        outp[:],
                                        xall[:,
                                             k * BT + g * GSZ + tt * 128:
                                             k * BT + g * GSZ + (tt + 1) * 128],
                                        wo[k][:, n2 * 512:(n2 + 1) * 512],
                                        start=(k == 0), stop=(k == 3))
                                nc.vector.tensor_copy(osb[:, ncol], outp[:])
                            if oh == 1:
                                sqp = ptrp.tile([128, 512], BF16, tag="trp")
                                nc.tensor.transpose(
                                    sqp[0:128, 0:1],
                                    ssq[0:1, tt * 128:(tt + 1) * 128],
                                    c_ibf[0:1, 0:1])
                                nc.vector.tensor_copy(
                                    osb[:, 1024:1025], sqp[:, 0:1])
                                nc.vector.memset(osb[:, 1025:hw], 0.0)
                            nc.sync.dma_start(
                                rs_in[g][tt * 128:(tt + 1) * 128,
                                         oh * 1024:oh * 1024 + hw], osb[:])
                    if _STOP == "nocc":
                        continue
                    nc.gpsimd.collective_compute(
                        "ReduceScatter", ALU.add, replica_groups=rg,
                        ins=[rs_in[g].opt()], outs=[rs_out[g].opt()])

                    # final gated-norm scale on own token shard
                    gsb = fin.tile([64, 1], BF16, tag="gsb")
                    nc.sync.dma_start(gsb[:], rs_out[g][:, 2048:2049])
                    gln = fin.tile([64, 1], FP32, tag="gln")
                    nc.scalar.activation(gln[:], gsb[:], AF.Ln,
                                         scale=1.0 / (2 * DM),
                                         bias=c_eps[0:64, :])
                    gcol = fin.tile([64, 1], FP32, tag="gcol")
                    nc.scalar.activation(gcol[:], gln[:], AF.Exp, scale=-0.5)
                    for hf in range(8):
                        cl = slice(hf * (DM // 8), (hf + 1) * (DM // 8))
                        ld = fin.tile([64, DM // 8], BF16, tag="ld",
                                      name=f"ld{g}_{hf}")
                        nc.sync.dma_start(ld[:], rs_out[g][:, cl])
                        fo = fin.tile([64, DM // 8], BF16, tag="fo",
                                      name=f"fo{g}_{hf}")
                        nc.vector.tensor_scalar_mul(fo[:], ld[:], gcol[:])
                        nc.sync.dma_start(out_rs[g * 64:(g + 1) * 64, cl],
                                          fo[:])

        except _StopBuild:
            pass
    nc.compile()
    return nc


def _get_built():
    global _BUILT
    if _BUILT is None:
        _BUILT = _build()
    return _BUILT


def kernel(**inputs):
    hs = np.ascontiguousarray(np.asarray(inputs["hidden_states"],
                                         dtype=np.float32))
    rd = np.ascontiguousarray(np.asarray(inputs["residual"], dtype=np.float32))
    B, L, Dm = hs.shape
    norm_w = np.asarray(inputs["norm_w"], dtype=np.float32)
    in_w = np.asarray(inputs["in_proj_w"], dtype=np.float32)
    conv_w = np.asarray(inputs["conv_w"], dtype=np.float32)
    conv_b = np.asarray(inputs["conv_b"], dtype=np.float32)
    A_log = np.asarray(inputs["A_log"], dtype=np.float32)
    D_param = np.asarray(inputs["D_param"], dtype=np.float32)
    dt_bias = np.asarray(inputs["dt_bias"], dtype=np.float32)
    gnw = np.asarray(inputs["gate_norm_w"], dtype=np.float32)
    out_w = np.asarray(inputs["out_proj_w"], dtype=np.float32)

    hid2 = hs.reshape(BT, DM)
    res2 = rd.reshape(BT, DM)
    Wn = in_w * norm_w[None, :]
    Wg = out_w * gnw[None, :]

    # select matrices for the wide decay matmul
    sel24n = np.zeros((24, NH * Q), np.float32)
    sel16p = np.zeros((16, NH * Q), np.float32)
    for lvl in range(3):
        for r in range(NH):
            sel24n[8 * lvl + r, r * Q:(r + 1) * Q] = -1.0
    for lvl in range(2):
        for r in range(NH):
            sel16p[8 * lvl + r, r * Q:(r + 1) * Q] = 1.0
    sidx = np.arange(Q)[:, None]
    tidx = np.arange(Q)[None, :]
    m0 = np.where(sidx > tidx, np.float32(-1e30), np.float32(0.0))
    m0t8 = np.tile(m0, (1, NH))
    wselm = np.zeros((NH, DI), np.float32)
    for r in range(NH):
        wselm[r, r * PD:(r + 1) * PD] = 1.0

    common = {
        "hid": hid2, "res": res2,
        "ones_col_bf": np.ones((128, 1), ml_dtypes.bfloat16),
        "i_bf": np.eye(128, dtype=ml_dtypes.bfloat16),
        "ones3": np.ones((3, 128), ml_dtypes.bfloat16),
        "sel24n": sel24n.astype(ml_dtypes.bfloat16),
        "sel16p": sel16p.astype(ml_dtypes.bfloat16),
        "m0t8": m0t8.astype(ml_dtypes.bfloat16),
        "wsel": wselm.astype(ml_dtypes.bfloat16),
    }

    in_maps = []
    for c in range(NCORES):
        rows = np.r_[4096 + 512 * c:4096 + 512 * (c + 1),
                     8192 + 128 * c:8192 + 128 * (c + 1),
                     9216 + 128 * c:9216 + 128 * (c + 1),
                     512 * c:512 * (c + 1),
                     10240 + 8 * c:10240 + 8 * (c + 1)]
        w_in_t = np.ascontiguousarray(Wn[rows, :].T).astype(ml_dtypes.bfloat16)
        w_out_t = np.ascontiguousarray(
            Wg[:, 512 * c:512 * (c + 1)].T).astype(ml_dtypes.bfloat16)
        crows = np.r_[512 * c:512 * (c + 1),
                      4096 + 128 * c:4096 + 128 * (c + 1),
                      5120 + 128 * c:5120 + 128 * (c + 1)]
        diag4 = np.zeros((128, 512), np.float32)
        for pi in range(4):
            dpair = np.repeat(D_param[8 * c + 2 * pi:8 * c + 2 * pi + 2], PD)
            diag4[:, pi * 128:(pi + 1) * 128] = np.diag(dpair)
        in_maps.append(dict(
            common,
            w_in_t=w_in_t,
            w_out_t=w_out_t,
            a_col=(-np.exp(A_log[8 * c:8 * (c + 1)])).reshape(8, 1)
                  .astype(np.float32),
            dtb_col=dt_bias[8 * c:8 * (c + 1)].reshape(8, 1).astype(np.float32),
            diag4=diag4.astype(ml_dtypes.bfloat16),
            convw=np.ascontiguousarray(
                conv_w[crows, :].reshape(6, 128, 4).transpose(1, 0, 2)
                .reshape(128, 24)).astype(np.float32),
            convb=np.ascontiguousarray(
                conv_b[crows].reshape(6, 128).T).astype(np.float32),
        ))

    nc = _get_built()
    res_k = bass_utils.run_bass_kernel_spmd(
        nc, in_maps, core_ids=list(range(NCORES)))
    global LAST_RESULTS
    LAST_RESULTS = res_k

    out = np.empty((BT, DM), np.float32)
    for c in range(NCORES):
        o = np.asarray(res_k.results[c]["out_rs"]).astype(np.float32)
        for g in range(NGRP):
            out[g * GSZ + c * 64:g * GSZ + (c + 1) * 64, :] = \
                o[g * 64:(g + 1) * 64, :]
    new_res = np.asarray(res_k.results[0]["new_res"]).astype(np.float32)
    return out.reshape(B, L, Dm), new_res.reshape(B, L, Dm)
